# revision 36
# baseline (speedup 1.0000x reference)
"""Trainium2 Bass kernel for multi-head attention returning (p_val, p_attn).

Full inputs: query/key/value [B=2, H=16, S=2048, D=128] fp32 (+ falsy m).
Sharding: B*H = 32 flat heads -> 4 heads per core across 8 NeuronCores
(data/head parallel, per the sharding hint); each core computes its full
SxS attention blocks independently; no collectives.

Host side casts q/k/v to fp16 (error budget ~1e-3 vs the 2048-term fp32
softmax; measured ~7e-4 end to end) and pre-transposes q/k to [d, s], so
all device loads are plain contiguous DMAs and the PE gets
fast-weight-load-eligible fp16 operands.

Per-core program, per head:
  - qT16/kT16 [128d, 2048s] and V loaded with contiguous DMAs.
  - Stage A (p_attn): per 128-row q-tile, two [128,1024] halves: 2 fp16
    matmuls -> scores PSUM; ScalarE exp (scale=1/sqrt(D), accum_out
    partial row-sum); DVE adds the halves, reciprocal, normalize,
    DMA out one [128, 2048] fp32 row-block.
  - Stage B (p_val): per 1024-wide q-chunk, for each of 16 k-chunks:
    scoresT [128k, 1024q] via 2 matmuls; exp -> pT fp16; 2 PV^T matmuls
    (lhsT = V chunk) accumulating pvT PSUM [128d, 1024q] over k.
    pvT is written out UNNORMALIZED in [d, q] layout (contiguous DMA);
    the host transposes and divides by the row sums.

Engine budget per core (measured): ScalarE ~282us (2 exp passes over
S*S), PE ~283us (QK^T twice - once per layout - plus PV^T; matmul +
serialized LDWEIGHTS), both >90% dense; HW exec ~304us. PSUM:
scores 2 banks + scoresT 2x2 + pvT 2. Rejected alternatives (measured
worse): PE-transposing P for the PV contraction (small-matmul overhead),
SBUF->SBUF DMA-xbar transposes of P (serializes, 1.03ms), fp32r
matmuls (4-byte self-loading weights, no FWL), N=1024 matmuls (ISA
reject), per-pair ldweights dedup (field ignored by walrus).
"""

import sys

sys.path.insert(0, "/opt/trn_rl_repo")

import math

import numpy as np

import concourse.bass as bass
import concourse.mybir as mybir
import concourse.tile as tile
from concourse import bacc
from concourse.bass_utils import run_bass_kernel_spmd

B, H, S, D = 2, 16, 2048, 128
NCORES = 8
HC = (B * H) // NCORES  # heads per core
P = 128
NQ = S // P  # 16 q-tiles per head
NK = S // P  # 16 k-chunks per head
QC = 512  # stage-B q-chunk width
NQC = S // QC
KB = 512  # stage-A scores matmul free-dim
NKB = S // KB
SCALE = 1.0 / math.sqrt(D)

f32 = mybir.dt.float32
f32r = mybir.dt.float32r
f16 = mybir.dt.float16
EXP = mybir.ActivationFunctionType.Exp
ts = bass.ts


def _build_program():
    nc = bacc.Bacc("TRN2", target_bir_lowering=False, debug=False, num_devices=NCORES)

    q_d = nc.dram_tensor("qT16", [HC, D, S], f16, kind="ExternalInput").ap()
    k_d = nc.dram_tensor("kT16", [HC, D, S], f16, kind="ExternalInput").ap()
    v_d = nc.dram_tensor("v16", [HC, S, D], f16, kind="ExternalInput").ap()
    pa_d = nc.dram_tensor("p_attn", [HC, S, S], f32, kind="ExternalOutput").ap()
    # p_val is written UNNORMALIZED in transposed [d, q] layout; the host
    # divides by rsums (written below) and transposes back.
    pv_d = nc.dram_tensor("p_val_t", [HC, NQC, D, QC], f32, kind="ExternalOutput").ap()
    rs_d = nc.dram_tensor("rsums", [HC, P, NQ], f32, kind="ExternalOutput").ap()

    with tile.TileContext(nc) as tc:
        with (
            tc.tile_pool(name="qkT", bufs=2) as qkT,
            tc.tile_pool(name="vp", bufs=2) as vp,
            tc.tile_pool(name="pp", bufs=3) as pp,
            tc.tile_pool(name="pnp", bufs=3) as pnp,
            tc.tile_pool(name="ptp", bufs=4) as ptp,
            tc.tile_pool(name="pvtp", bufs=3) as pvtp,
            tc.tile_pool(name="stat", bufs=2) as stat,
            tc.tile_pool(name="ps_sc", bufs=1, space="PSUM") as ps_sc,
            tc.tile_pool(name="ps_st", bufs=2, space="PSUM") as ps_st,
            tc.tile_pool(name="ps_pvt", bufs=1, space="PSUM") as ps_pvt,
        ):
            for h in range(HC):
                # ---- load qT/kT [128d, S]: pre-transposed on the host,
                # so these are plain contiguous DMAs (kT first - the first
                # exp waits on it; V last - only stage B needs it).
                qT16 = qkT.tile([P, S], f16, tag="qT16")
                kT16 = qkT.tile([P, S], f16, tag="kT16")
                if h == 0:
                    # first head's loads gate the whole pipeline: split along
                    # the free dim (full 128 partitions per chunk) so four
                    # queues carry them in parallel
                    for c in range(4):
                        nc.sync.dma_start(
                            kT16[:, ts(c, 512)], k_d[h, :, ts(c, 512)]
                        )
                    for c in range(4):
                        nc.sync.dma_start(
                            qT16[:, ts(c, 512)], q_d[h, :, ts(c, 512)]
                        )
                else:
                    nc.sync.dma_start(kT16[:], k_d[h])
                    nc.sync.dma_start(qT16[:], q_d[h])
                v16 = vp.tile([P, NK, P], f16, tag="v16")
                nc.sync.dma_start(
                    v16[:], v_d[h].rearrange("(ko p) d -> p ko d", p=P)
                )

                rs2 = stat.tile([P, NQ, 2], f32, tag="rs2")
                rs = stat.tile([P, NQ], f32, tag="rs")
                rc = stat.tile([P, NQ], f32, tag="rc")

                # ---- stage A: p_attn ----
                # scores PSUM is a [128, 1024] half-row; exp runs 1024-wide
                # (amortizes the fixed ACT per-call cost) with one partial
                # row-sum per half, added on the DVE.
                for qi in range(NQ):
                    p_sb = pp.tile([P, S], f32, tag="p")
                    for kh in range(2):
                        sc = ps_sc.tile([P, 1024], f32, tag="sc")
                        for kb in range(2):
                            nc.tensor.matmul(
                                sc[:, ts(kb, KB)],
                                qT16[:, ts(qi, P)],
                                kT16[:, ts(kh * 2 + kb, KB)],
                                start=True,
                                stop=True,
                            )
                        nc.scalar.activation(
                            p_sb[:, ts(kh, 1024)],
                            sc[:],
                            EXP,
                            scale=SCALE,
                            accum_out=rs2[:, qi, kh : kh + 1],
                        )
                    nc.vector.tensor_add(
                        rs[:, qi : qi + 1],
                        rs2[:, qi, 0:1],
                        rs2[:, qi, 1:2],
                    )
                    nc.vector.reciprocal(rc[:, qi : qi + 1], rs[:, qi : qi + 1])
                    p_nm = pnp.tile([P, S], f32, tag="pn")
                    nc.vector.tensor_scalar_mul(
                        p_nm[:], p_sb[:], rc[:, qi : qi + 1]
                    )
                    nc.sync.dma_start(pa_d[h, ts(qi, P), :], p_nm[:])
                nc.sync.dma_start(rs_d[h], rs[:])

                # ---- stage B: p_val ----
                # 1024-wide q chunks: scoresT/exp/PV all 1024 wide (pairs of
                # 512-wide matmuls; the moving operand is capped at 512).
                for qp in range(NQC // 2):
                    pvt = ps_pvt.tile([P, 1024], f32, tag="pvt")
                    for kb in range(NK):
                        st = ps_st.tile([P, 1024], f32, tag="st")
                        for half in range(2):
                            nc.tensor.matmul(
                                st[:, ts(half, QC)],
                                kT16[:, ts(kb, P)],
                                qT16[:, ts(qp * 2 + half, QC)],
                                start=True,
                                stop=True,
                            )
                        pt = ptp.tile([P, 1024], f16, tag="pt")
                        nc.scalar.activation(pt[:], st[:], EXP, scale=SCALE)
                        for half in range(2):
                            nc.tensor.matmul(
                                pvt[:, ts(half, QC)],
                                v16[:, kb, :],
                                pt[:, ts(half, QC)],
                                start=(kb == 0),
                                stop=(kb == NK - 1),
                            )
                    pvt_s = pvtp.tile([P, 1024], f32, tag="pvts")
                    nc.vector.tensor_copy(pvt_s[:], pvt[:])
                    nc.sync.dma_start(
                        pv_d[h, ts(qp, 2)].rearrange("c d q -> d c q"), pvt_s[:]
                    )

    nc.compile()
    return nc


_NC_CACHE = {}


def _get_program():
    if "nc" not in _NC_CACHE:
        _NC_CACHE["nc"] = _build_program()
    return _NC_CACHE["nc"]


def kernel(query, key, value, m=None, _trace=False):
    qf = np.asarray(query, dtype=np.float32).reshape(B * H, S, D).astype(np.float16)
    kf = np.asarray(key, dtype=np.float32).reshape(B * H, S, D).astype(np.float16)
    vf = np.asarray(value, dtype=np.float32).reshape(B * H, S, D).astype(np.float16)
    qtf = np.ascontiguousarray(qf.transpose(0, 2, 1))  # [BH, D, S]
    ktf = np.ascontiguousarray(kf.transpose(0, 2, 1))

    nc = _get_program()
    in_maps = [
        {
            "qT16": qtf[c * HC : (c + 1) * HC],
            "kT16": ktf[c * HC : (c + 1) * HC],
            "v16": np.ascontiguousarray(vf[c * HC : (c + 1) * HC]),
        }
        for c in range(NCORES)
    ]
    res = run_bass_kernel_spmd(nc, in_maps, list(range(NCORES)), trace=_trace)

    p_attn = np.empty((B * H, S, S), dtype=np.float32)
    p_val = np.empty((B * H, S, D), dtype=np.float32)
    for c in range(NCORES):
        p_attn[c * HC : (c + 1) * HC] = res.results[c]["p_attn"]
        # p_val_t: [HC, NQC, D, QC] unnormalized -> [HC, S, D] normalized
        pvt = res.results[c]["p_val_t"].transpose(0, 1, 3, 2).reshape(HC, S, D)
        # rsums: [HC, P, NQ] -> sums[h, qi*P + p] = rsums[h, p, qi]
        sums = res.results[c]["rsums"].transpose(0, 2, 1).reshape(HC, S, 1)
        p_val[c * HC : (c + 1) * HC] = pvt / sums

    p_attn = p_attn.reshape(B, H, S, S)
    p_val = p_val.reshape(B, H, S, D)
    if _trace:
        kernel._last_results = res
    return (p_val, p_attn)


# revision 38
# speedup vs baseline: 1.0031x; 1.0031x over previous
"""Trainium2 Bass kernel for multi-head attention returning (p_val, p_attn).

Full inputs: query/key/value [B=2, H=16, S=2048, D=128] fp32 (+ falsy m).
Sharding: B*H = 32 flat heads -> 4 heads per core across 8 NeuronCores
(data/head parallel, per the sharding hint); each core computes its full
SxS attention blocks independently; no collectives.

Host side casts q/k/v to fp16 (error budget ~1e-3 vs the 2048-term fp32
softmax; measured ~7e-4 end to end) and pre-transposes q/k to [d, s], so
all device loads are plain contiguous DMAs and the PE gets
fast-weight-load-eligible fp16 operands.

Per-core program, per head:
  - qT16/kT16 [128d, 2048s] and V loaded with contiguous DMAs.
  - Stage A (p_attn): per 128-row q-tile, two [128,1024] halves: 2 fp16
    matmuls -> scores PSUM; ScalarE exp (scale=1/sqrt(D), accum_out
    partial row-sum); DVE adds the halves, reciprocal, normalize,
    DMA out one [128, 2048] fp32 row-block.
  - Stage B (p_val): per 1024-wide q-chunk, for each of 16 k-chunks:
    scoresT [128k, 1024q] via 2 matmuls; exp -> pT fp16; 2 PV^T matmuls
    (lhsT = V chunk) accumulating pvT PSUM [128d, 1024q] over k.
    pvT is written out UNNORMALIZED in [d, q] layout (contiguous DMA);
    the host transposes and divides by the row sums.

Engine budget per core (measured): ScalarE ~282us (2 exp passes over
S*S), PE ~283us (QK^T twice - once per layout - plus PV^T; matmul +
serialized LDWEIGHTS), both >90% dense; HW exec ~304us. PSUM:
scores 2 banks + scoresT 2x2 + pvT 2. Rejected alternatives (measured
worse): PE-transposing P for the PV contraction (small-matmul overhead),
SBUF->SBUF DMA-xbar transposes of P (serializes, 1.03ms), fp32r
matmuls (4-byte self-loading weights, no FWL), N=1024 matmuls (ISA
reject), per-pair ldweights dedup (field ignored by walrus).
"""

import sys

sys.path.insert(0, "/opt/trn_rl_repo")

import math

import numpy as np

import concourse.bass as bass
import concourse.mybir as mybir
import concourse.tile as tile
from concourse import bacc
from concourse.bass_utils import run_bass_kernel_spmd

B, H, S, D = 2, 16, 2048, 128
NCORES = 8
HC = (B * H) // NCORES  # heads per core
P = 128
NQ = S // P  # 16 q-tiles per head
NK = S // P  # 16 k-chunks per head
QC = 512  # stage-B q-chunk width
NQC = S // QC
KB = 512  # stage-A scores matmul free-dim
NKB = S // KB
SCALE = 1.0 / math.sqrt(D)

f32 = mybir.dt.float32
f32r = mybir.dt.float32r
f16 = mybir.dt.float16
EXP = mybir.ActivationFunctionType.Exp
ts = bass.ts


def _build_program():
    nc = bacc.Bacc("TRN2", target_bir_lowering=False, debug=False, num_devices=NCORES)

    q_d = nc.dram_tensor("qT16", [HC, D, S], f16, kind="ExternalInput").ap()
    k_d = nc.dram_tensor("kT16", [HC, D, S], f16, kind="ExternalInput").ap()
    v_d = nc.dram_tensor("v16", [HC, S, D], f16, kind="ExternalInput").ap()
    pa_d = nc.dram_tensor("p_attn", [HC, S, S], f32, kind="ExternalOutput").ap()
    # p_val is written UNNORMALIZED in transposed [d, q] layout; the host
    # divides by rsums (written below) and transposes back.
    pv_d = nc.dram_tensor("p_val_t", [HC, NQC, D, QC], f32, kind="ExternalOutput").ap()
    rs_d = nc.dram_tensor("rsums", [HC, P, NQ], f32, kind="ExternalOutput").ap()

    with tile.TileContext(nc) as tc:
        with (
            tc.tile_pool(name="qkT", bufs=2) as qkT,
            tc.tile_pool(name="vp", bufs=2) as vp,
            tc.tile_pool(name="pp", bufs=3) as pp,
            tc.tile_pool(name="pnp", bufs=3) as pnp,
            tc.tile_pool(name="ptp", bufs=4) as ptp,
            tc.tile_pool(name="pvtp", bufs=3) as pvtp,
            tc.tile_pool(name="stat", bufs=2) as stat,
            tc.tile_pool(name="ps_sc", bufs=1, space="PSUM") as ps_sc,
            tc.tile_pool(name="ps_st", bufs=2, space="PSUM") as ps_st,
            tc.tile_pool(name="ps_pvt", bufs=1, space="PSUM") as ps_pvt,
        ):
            for h in range(HC):
                # ---- load qT/kT [128d, S]: pre-transposed on the host,
                # so these are plain contiguous DMAs (kT first - the first
                # exp waits on it; V last - only stage B needs it).
                # split q/k operands into two 1024-wide tiles so the
                # first matmuls depend on only half the input loads
                # (tile-granular deps otherwise gate them on the full 2048)
                kTh = [
                    qkT.tile([P, 1024], f16, tag=f"kT{i}", name=f"kT{i}_{h}")
                    for i in range(2)
                ]
                qTh = [
                    qkT.tile([P, 1024], f16, tag=f"qT{i}", name=f"qT{i}_{h}")
                    for i in range(2)
                ]
                for i in range(2):
                    nc.sync.dma_start(kTh[i][:], k_d[h, :, ts(i, 1024)])
                for i in range(2):
                    nc.sync.dma_start(qTh[i][:], q_d[h, :, ts(i, 1024)])
                v16 = vp.tile([P, NK, P], f16, tag="v16")
                nc.sync.dma_start(
                    v16[:], v_d[h].rearrange("(ko p) d -> p ko d", p=P)
                )

                rs2 = stat.tile([P, NQ, 2], f32, tag="rs2")
                rs = stat.tile([P, NQ], f32, tag="rs")
                rc = stat.tile([P, NQ], f32, tag="rc")

                # ---- stage A: p_attn ----
                # scores PSUM is a [128, 1024] half-row; exp runs 1024-wide
                # (amortizes the fixed ACT per-call cost) with one partial
                # row-sum per half, added on the DVE.
                for qi in range(NQ):
                    p_sb = pp.tile([P, S], f32, tag="p")
                    for kh in range(2):
                        sc = ps_sc.tile([P, 1024], f32, tag="sc")
                        for kb in range(2):
                            nc.tensor.matmul(
                                sc[:, ts(kb, KB)],
                                qTh[qi // 8][:, ts(qi % 8, P)],
                                kTh[kh][:, ts(kb, KB)],
                                start=True,
                                stop=True,
                            )
                        nc.scalar.activation(
                            p_sb[:, ts(kh, 1024)],
                            sc[:],
                            EXP,
                            scale=SCALE,
                            accum_out=rs2[:, qi, kh : kh + 1],
                        )
                    nc.vector.tensor_add(
                        rs[:, qi : qi + 1],
                        rs2[:, qi, 0:1],
                        rs2[:, qi, 1:2],
                    )
                    nc.vector.reciprocal(rc[:, qi : qi + 1], rs[:, qi : qi + 1])
                    p_nm = pnp.tile([P, S], f32, tag="pn")
                    nc.vector.tensor_scalar_mul(
                        p_nm[:], p_sb[:], rc[:, qi : qi + 1]
                    )
                    nc.sync.dma_start(pa_d[h, ts(qi, P), :], p_nm[:])
                nc.sync.dma_start(rs_d[h], rs[:])

                # ---- stage B: p_val ----
                # 1024-wide q chunks: scoresT/exp/PV all 1024 wide (pairs of
                # 512-wide matmuls; the moving operand is capped at 512).
                for qp in range(NQC // 2):
                    pvt = ps_pvt.tile([P, 1024], f32, tag="pvt")
                    for kb in range(NK):
                        st = ps_st.tile([P, 1024], f32, tag="st")
                        for half in range(2):
                            nc.tensor.matmul(
                                st[:, ts(half, QC)],
                                kTh[kb // 8][:, ts(kb % 8, P)],
                                qTh[qp][:, ts(half, QC)],
                                start=True,
                                stop=True,
                            )
                        pt = ptp.tile([P, 1024], f16, tag="pt")
                        nc.scalar.activation(pt[:], st[:], EXP, scale=SCALE)
                        for half in range(2):
                            nc.tensor.matmul(
                                pvt[:, ts(half, QC)],
                                v16[:, kb, :],
                                pt[:, ts(half, QC)],
                                start=(kb == 0),
                                stop=(kb == NK - 1),
                            )
                    pvt_s = pvtp.tile([P, 1024], f32, tag="pvts")
                    nc.vector.tensor_copy(pvt_s[:], pvt[:])
                    nc.sync.dma_start(
                        pv_d[h, ts(qp, 2)].rearrange("c d q -> d c q"), pvt_s[:]
                    )

    nc.compile()
    return nc


_NC_CACHE = {}


def _get_program():
    if "nc" not in _NC_CACHE:
        _NC_CACHE["nc"] = _build_program()
    return _NC_CACHE["nc"]


def kernel(query, key, value, m=None, _trace=False):
    qf = np.asarray(query, dtype=np.float32).reshape(B * H, S, D).astype(np.float16)
    kf = np.asarray(key, dtype=np.float32).reshape(B * H, S, D).astype(np.float16)
    vf = np.asarray(value, dtype=np.float32).reshape(B * H, S, D).astype(np.float16)
    qtf = np.ascontiguousarray(qf.transpose(0, 2, 1))  # [BH, D, S]
    ktf = np.ascontiguousarray(kf.transpose(0, 2, 1))

    nc = _get_program()
    in_maps = [
        {
            "qT16": qtf[c * HC : (c + 1) * HC],
            "kT16": ktf[c * HC : (c + 1) * HC],
            "v16": np.ascontiguousarray(vf[c * HC : (c + 1) * HC]),
        }
        for c in range(NCORES)
    ]
    res = run_bass_kernel_spmd(nc, in_maps, list(range(NCORES)), trace=_trace)

    p_attn = np.empty((B * H, S, S), dtype=np.float32)
    p_val = np.empty((B * H, S, D), dtype=np.float32)
    for c in range(NCORES):
        p_attn[c * HC : (c + 1) * HC] = res.results[c]["p_attn"]
        # p_val_t: [HC, NQC, D, QC] unnormalized -> [HC, S, D] normalized
        pvt = res.results[c]["p_val_t"].transpose(0, 1, 3, 2).reshape(HC, S, D)
        # rsums: [HC, P, NQ] -> sums[h, qi*P + p] = rsums[h, p, qi]
        sums = res.results[c]["rsums"].transpose(0, 2, 1).reshape(HC, S, 1)
        p_val[c * HC : (c + 1) * HC] = pvt / sums

    p_attn = p_attn.reshape(B, H, S, S)
    p_val = p_val.reshape(B, H, S, D)
    if _trace:
        kernel._last_results = res
    return (p_val, p_attn)
